# revision 1
# baseline (speedup 1.0000x reference)
"""GAT+LSTM fused kernel for 8 trn2 NeuronCores.

- Output depends only on batch row T-1=11 of the reference LSTM (ys[:, -1]),
  so only GAT outputs for nodes [110000, 120000) ("live" nodes) are needed.
- Edges sharded by src-range across 8 cores (only edges with a live dst);
  self-loops are injected as ordinary edges (their edge_attr slot is filled
  on-device with mean(edge_attr) after a tiny AllReduce).
- Per-edge rows fetched with dma_gather from per-core blocked node tables;
  segment softmax without max-subtraction (numerically safe here).
- Aggregation: host groups each core's edges by destination bucket
  (dst >> 7), exactly 2 chunks of 128 per bucket (canonical schedule, same
  for every core); on-chip each chunk builds a one-hot (dst == iota) matrix
  and a PE matmul accumulates messages into the bucket's PSUM tile.
- Partial accumulators combined with ReduceScatter; normalize + ReLU +
  transpose; AllGather; LSTM input projection.
- LSTM solved by fixed-point iteration (ITERS full-sequence passes) with
  tensor_tensor_scan for the cell recurrence; converges to the exact serial
  recurrence to f32 noise in ~12 passes.
"""
import os
import numpy as np

import concourse.bass as bass
import concourse.bacc as bacc
import concourse.tile as tile
from concourse import mybir
from concourse.bass_utils import run_bass_kernel_spmd
from concourse.masks import make_identity
from contextlib import ExitStack

dt = mybir.dt
F32 = dt.float32
I16 = dt.int16
AF = mybir.ActivationFunctionType
ALU = mybir.AluOpType

T, N, F_IN = 12, 10000, 64
HEADS, C, HID = 4, 32, 32
E, TN = 1_000_000, 120_000
NCORES = 8
NSH = TN // NCORES              # 15000 nodes per x-shard
NBLK = 118                      # main node-table blocks (118*128 = 15104)
NSHP = NBLK * 128
D0 = (T - 1) * N
DN = N
DBLK = 80                       # live-node buckets (80*128 = 10240)
DNP = DBLK * 128
DSL = DNP // NCORES             # 1280 live nodes owned per core
SBLK = 10                       # appended self-node blocks (10*128 = 1280)
NROWS = NSHP + SBLK * 128       # 16384 rows in h/asrc tables
BSLOT = 256                     # canonical slots per bucket (2 chunks)
NCH = DBLK * 2                  # 160 chunks
CAP = NCH * 128                 # 20480
NPASS = 2                       # edge phase in 2 passes of 80 chunks
NT = DNP
SC = 2048
NSC = NT // SC
ITERS = 8
LEAK = 0.2

_CACHE = {}


def _wrap16(idx, cap):
    out = np.zeros((16, cap // 16), np.int16)
    j = np.arange(len(idx))
    out[j % 16, j // 16] = np.asarray(idx).astype(np.int16)
    return np.tile(out, (8, 1))


def _chunkify(vals, cap, fill):
    out = np.full(cap, fill, np.float32)
    out[:len(vals)] = vals
    return np.ascontiguousarray(out.reshape(cap // 128, 128).T)


def _row_main(n):
    return (n % 128) * NBLK + n // 128


def _row_self(c):
    return NSHP + (c % 128) * SBLK + c // 128


def _row_ad(n):
    return (n % 128) * DBLK + n // 128


def _prep_host(inputs):
    x = np.ascontiguousarray(np.asarray(inputs["x_seq"], np.float32).reshape(TN, F_IN))
    ei = np.asarray(inputs["edge_index"])
    ea = np.asarray(inputs["edge_attr"], np.float32)[:, 0]
    W_gat = np.asarray(inputs["W_gat"], np.float32)
    att_src = np.asarray(inputs["att_src"], np.float32)
    att_dst = np.asarray(inputs["att_dst"], np.float32)
    att_edge = np.asarray(inputs["att_edge"], np.float32)
    W_edge = np.asarray(inputs["W_edge"], np.float32)
    gat_bias = np.asarray(inputs["gat_bias"], np.float32)
    W_ih = np.asarray(inputs["W_ih"], np.float32)
    W_hh = np.asarray(inputs["W_hh"], np.float32)
    b = np.asarray(inputs["b_ih"], np.float32) + np.asarray(inputs["b_hh"], np.float32)
    W_fc = np.asarray(inputs["W_fc"], np.float32)
    b_fc = np.asarray(inputs["b_fc"], np.float32)

    A_src = np.zeros((HEADS * C, HEADS), np.float32)
    A_dst = np.zeros((HEADS * C, HEADS), np.float32)
    for h in range(HEADS):
        A_src[h * C:(h + 1) * C, h] = att_src[h]
        A_dst[h * C:(h + 1) * C, h] = att_dst[h]
    Wa8 = np.concatenate([W_gat @ A_src, W_gat @ A_dst], axis=1)
    kap = np.array([np.dot(W_edge[0, h * C:(h + 1) * C], att_edge[h])
                    for h in range(HEADS)], np.float32)
    kap_rep = np.broadcast_to(kap, (128, HEADS)).copy()
    gbias_rep = np.broadcast_to(gat_bias, (128, HEADS * C)).copy()
    iota128 = np.broadcast_to(np.arange(128, dtype=np.float32), (128, 128)).copy()
    perm = np.concatenate([np.arange(32, 64), np.arange(0, 32),
                           np.arange(96, 128), np.arange(64, 96)])
    WihT = np.ascontiguousarray(W_ih[perm].T)
    WhhT = np.ascontiguousarray(W_hh[perm].T)
    br = np.ascontiguousarray(b[perm].reshape(128, 1))

    src, dst = ei[0].astype(np.int64), ei[1].astype(np.int64)
    live = (dst >= D0) & (dst < D0 + DN)
    core_of = src // NSH
    xTD = np.zeros((F_IN, DNP), np.float32)
    xTD[:, :DN] = x[D0:D0 + DN].T
    ZN, ZD = NSH, DN + 100      # zero-content pad nodes (shard / live)
    in_maps = []
    ea_all = ea.reshape(NCORES, E // NCORES)
    for k in range(NCORES):
        m = live & (core_of == k)
        sL = src[m] - k * NSH
        dL = dst[m] - D0
        eav = ea[m]
        own = np.arange(DNP).reshape(DBLK, 8, 16)[:, k, :].ravel()
        own_cols = 16 * (own // 128) + (own % 128 - 16 * k)
        o2 = np.argsort(own_cols)
        own = own[o2]                       # xTDS column c <-> live node own[c]
        xTDS = np.ascontiguousarray(xTD[:, own])
        bflat = dL >> 7
        hs_idx = np.full(CAP, _row_main(ZN), np.int64)
        ad_idx = np.full(CAP, _row_ad(ZD), np.int64)
        eac = np.zeros(CAP, np.float32)
        dstf = np.full(CAP, -1.0, np.float32)
        selfm = np.zeros(CAP, np.float32)
        for bkt in range(DBLK):
            sel = np.nonzero(bflat == bkt)[0]
            sn = own[(own >= 128 * bkt) & (own < 128 * (bkt + 1))]
            cn = 16 * (sn // 128) + (sn % 128 - 16 * k)
            nb = len(sel) + len(sn)
            assert nb <= BSLOT, f"core {k} bucket {bkt}: {nb} > {BSLOT}"
            o = bkt * BSLOT
            ne = len(sel)
            hs_idx[o:o + ne] = _row_main(sL[sel])
            ad_idx[o:o + ne] = _row_ad(dL[sel])
            dstf[o:o + ne] = dL[sel] - 128 * bkt
            eac[o:o + ne] = eav[sel]
            hs_idx[o + ne:o + nb] = _row_self(cn)
            ad_idx[o + ne:o + nb] = _row_ad(sn)
            dstf[o + ne:o + nb] = sn - 128 * bkt
            selfm[o + ne:o + nb] = 1.0
        xT = np.zeros((F_IN, NSHP), np.float32)
        xT[:, :NSH] = x[k * NSH:(k + 1) * NSH].T
        eaF = np.zeros((128, 980), np.float32)
        ch = ea_all[k]
        j2 = np.arange(len(ch))
        eaF[j2 % 128, j2 // 128] = ch
        in_maps.append({
            "xT": xT, "xTD": xTD, "xTDS": xTDS,
            "eaC": _chunkify(eac, CAP, 0.0),
            "dstF": _chunkify(dstf, CAP, -1.0),
            "selfM": _chunkify(selfm, CAP, 0.0),
            "eaF": eaF,
            "hsI": np.ascontiguousarray(
                hs_idx.reshape(NCH, 128).T).astype(np.int32),
            "adI": np.ascontiguousarray(
                ad_idx.reshape(NCH, 128).T).astype(np.int32),
            "Wgat": W_gat, "Wa8": Wa8, "kap": kap_rep, "gbias": gbias_rep,
            "iota": iota128,
            "Wih": WihT, "Whh": WhhT, "br": br,
            "Wfc": np.ascontiguousarray(W_fc.reshape(HID, 1)),
            "bfc": np.ascontiguousarray(b_fc.reshape(1, 1)),
        })
    return in_maps


def _build_nc(debug=False):
    STAGE = int(os.environ.get("KSTAGE", "99"))
    nc = bacc.Bacc("TRN2", target_bir_lowering=False, debug=False,
                   num_devices=NCORES)
    g = lambda n, s, d=F32: nc.dram_tensor(n, s, d, kind="ExternalInput").ap()
    xT = g("xT", [F_IN, NSHP]); xTD = g("xTD", [F_IN, DNP])
    xTDS = g("xTDS", [F_IN, DSL])
    eaC = g("eaC", [128, NCH]); dstF = g("dstF", [128, NCH])
    selfM = g("selfM", [128, NCH]); eaF = g("eaF", [128, 980])
    hsI = g("hsI", [128, NCH], dt.int32); adI = g("adI", [128, NCH], dt.int32)
    Wgat = g("Wgat", [F_IN, 128]); Wa8 = g("Wa8", [F_IN, 8])
    kap = g("kap", [128, HEADS]); gbias = g("gbias", [128, 128])
    iota = g("iota", [128, 128])
    Wih = g("Wih", [128, 128]); Whh = g("Whh", [HID, 128])
    br = g("br", [128, 1]); Wfc = g("Wfc", [HID, 1]); bfc = g("bfc", [1, 1])
    out = nc.dram_tensor("out", [1, NT], F32, kind="ExternalOutput").ap()
    if debug:
        dbg_gat = nc.dram_tensor("dbg_gat", [128, DSL], F32, kind="ExternalOutput").ap()
        dbg_gx = nc.dram_tensor("dbg_gx", [128, NT], F32, kind="ExternalOutput").ap()
        dbg_h = nc.dram_tensor("dbg_h", [HID, NT], F32, kind="ExternalOutput").ap()

    h_tbl = nc.dram_tensor("h_tbl", [NROWS, 192], F32).ap()
    ad_tbl = nc.dram_tensor("ad_tbl", [DNP, 64], F32).ap()
    acc_tbl = nc.dram_tensor("acc_tbl", [128, DBLK * 132], F32).ap()
    ea_in = nc.dram_tensor("ea_in", [128, 1], F32).ap()
    ea_out = nc.dram_tensor("ea_out", [128, 1], F32, addr_space="Shared").ap()
    rs_out = nc.dram_tensor("rs_out", [16, DBLK * 132], F32).ap()
    gat_blob = nc.dram_tensor("gat_blob", [128, DSL], F32).ap()
    gat_full = nc.dram_tensor("gat_full", [NCORES, 128, DSL], F32,
                              addr_space="Shared").ap()
    RG = [list(range(NCORES))]

    with tile.TileContext(nc) as tc, ExitStack() as top:
        const = top.enter_context(tc.tile_pool(name="const", bufs=1))
        ident = const.tile([128, 128], F32)
        make_identity(nc, ident[:])
        wgat_t = const.tile([F_IN, 128], F32); nc.sync.dma_start(wgat_t[:], Wgat[:])
        wa8_t = const.tile([F_IN, 8], F32); nc.sync.dma_start(wa8_t[:], Wa8[:])
        kap_t = const.tile([128, HEADS], F32); nc.sync.dma_start(kap_t[:], kap[:])
        gb_t = const.tile([128, 128], F32); nc.sync.dma_start(gb_t[:], gbias[:])
        iota_t = const.tile([128, 128], F32); nc.sync.dma_start(iota_t[:], iota[:])
        whh_t = const.tile([HID, 128], F32); nc.sync.dma_start(whh_t[:], Whh[:])
        wih_t = const.tile([128, 128], F32); nc.sync.dma_start(wih_t[:], Wih[:])
        br_t = const.tile([128, 1], F32); nc.sync.dma_start(br_t[:], br[:])
        wfc_t = const.tile([HID, 1], F32); nc.sync.dma_start(wfc_t[:], Wfc[:])
        bfc_t = const.tile([1, 1], F32); nc.sync.dma_start(bfc_t[:], bfc[:])
        meanr = const.tile([128, 1], F32)

        # ---------- A1: mean(edge_attr) via AllReduce + PE broadcast -------
        with ExitStack() as ph:
            sbm = ph.enter_context(tc.tile_pool(name="sbm", bufs=1))
            psm = ph.enter_context(tc.tile_pool(name="psm", bufs=1, space="PSUM"))
            eaf_t = sbm.tile([128, 980], F32)
            nc.sync.dma_start(eaf_t[:], eaF[:])
            eap = sbm.tile([128, 1], F32)
            nc.vector.tensor_reduce(eap[:], eaf_t[:], mybir.AxisListType.X, ALU.add)
            nc.sync.dma_start(ea_in[:], eap[:])
            nc.gpsimd.collective_compute("AllReduce", ALU.add, replica_groups=RG,
                                         ins=[ea_in[:]], outs=[ea_out[:]])
            eao_t = sbm.tile([128, 1], F32)
            nc.sync.dma_start(eao_t[:], ea_out[:])
            onc = sbm.tile([128, 1], F32)
            nc.gpsimd.memset(onc[:], 1.0)
            ps1 = psm.tile([1, 1], F32, space="PSUM", tag="ps1")
            nc.tensor.matmul(ps1[:], lhsT=eao_t[:], rhs=onc[:], start=True, stop=True)
            eas = sbm.tile([1, 1], F32)
            nc.scalar.mul(eas[:], ps1[:], 1.0 / E)
            onr = sbm.tile([1, 128], F32)
            nc.gpsimd.memset(onr[:], 1.0)
            ps2 = psm.tile([128, 1], F32, space="PSUM", tag="ps2")
            nc.tensor.matmul(ps2[:], lhsT=onr[:], rhs=eas[:], start=True, stop=True)
            nc.vector.tensor_copy(meanr[:], ps2[:])

        if STAGE >= 1:
            # ---------- A2/A3: blocked node tables ----------
            with ExitStack() as ph:
                sba = ph.enter_context(tc.tile_pool(name="sba", bufs=1))
                psa = ph.enter_context(tc.tile_pool(name="psa", bufs=4, space="PSUM"))
                psp = ph.enter_context(tc.tile_pool(name="psp", bufs=1, space="PSUM"))
                xt_t = sba.tile([F_IN, NSHP], F32)
                nc.sync.dma_start(xt_t[:], xT[:])
                HS = sba.tile([128, NBLK * 192], F32)
                HSv = HS[:].rearrange("p (j w) -> p j w", w=192)
                pack_a = psp.tile([128, NBLK * 4], F32, space="PSUM", tag="pka")
                for j in range(NBLK):
                    ph_ = psa.tile([128, 128], F32, space="PSUM", tag="ph")
                    nc.tensor.matmul(ph_[:], lhsT=xt_t[:, j * 128:(j + 1) * 128],
                                     rhs=wgat_t[:], start=True, stop=True)
                    nc.vector.tensor_copy(HSv[:, j, 0:128], ph_[:])
                    nc.tensor.matmul(pack_a[:, j * 4:(j + 1) * 4],
                                     lhsT=xt_t[:, j * 128:(j + 1) * 128],
                                     rhs=wa8_t[:, 0:4], start=True, stop=True)
                nc.vector.tensor_copy(HSv[:, :, 128:132],
                                      pack_a[:].rearrange("p (j w) -> p j w", w=4))
                nc.sync.dma_start(h_tbl[0:NSHP, :].rearrange("(p j) w -> p (j w)", p=128),
                                  HS[:])
            with ExitStack() as ph:
                sbb = ph.enter_context(tc.tile_pool(name="sbb", bufs=1))
                psb = ph.enter_context(tc.tile_pool(name="psb", bufs=4, space="PSUM"))
                psq = ph.enter_context(tc.tile_pool(name="psq", bufs=1, space="PSUM"))
                xtd_t = sbb.tile([F_IN, DNP], F32)
                nc.sync.dma_start(xtd_t[:], xTD[:])
                SAD = sbb.tile([128, DBLK * 64], F32)
                nc.gpsimd.memset(SAD[:], 0.0)
                pack_d = psq.tile([128, DBLK * 4], F32, space="PSUM", tag="pkd")
                for j in range(DBLK):
                    nc.tensor.matmul(pack_d[:, j * 4:(j + 1) * 4],
                                     lhsT=xtd_t[:, j * 128:(j + 1) * 128],
                                     rhs=wa8_t[:, 4:8], start=True, stop=True)
                SADv = SAD[:].rearrange("p (j w) -> p j w", w=64)
                nc.vector.tensor_copy(SADv[:, :, 0:4],
                                      pack_d[:].rearrange("p (j w) -> p j w", w=4))
                nc.sync.dma_start(ad_tbl[:].rearrange("(p j) w -> p (j w)", p=128),
                                  SAD[:])
                # appended self-node rows (from xTDS)
                xts_t = sbb.tile([F_IN, DSL], F32)
                nc.sync.dma_start(xts_t[:], xTDS[:])
                HS2 = sbb.tile([128, SBLK * 192], F32)
                HS2v = HS2[:].rearrange("p (j w) -> p j w", w=192)
                pack_s = psq.tile([128, SBLK * 4], F32, space="PSUM", tag="pks")
                for j in range(SBLK):
                    ph2 = psb.tile([128, 128], F32, space="PSUM", tag="ph2")
                    nc.tensor.matmul(ph2[:], lhsT=xts_t[:, j * 128:(j + 1) * 128],
                                     rhs=wgat_t[:], start=True, stop=True)
                    nc.vector.tensor_copy(HS2v[:, j, 0:128], ph2[:])
                    nc.tensor.matmul(pack_s[:, j * 4:(j + 1) * 4],
                                     lhsT=xts_t[:, j * 128:(j + 1) * 128],
                                     rhs=wa8_t[:, 0:4], start=True, stop=True)
                nc.vector.tensor_copy(HS2v[:, :, 128:132],
                                      pack_s[:].rearrange("p (j w) -> p j w", w=4))
                nc.sync.dma_start(
                    h_tbl[NSHP:NROWS, :].rearrange("(p j) w -> p (j w)", p=128), HS2[:])

        if STAGE >= 2:
            # ---------- A5: edge phase, 2 passes of 80 chunks ----------
            PC = NCH // NPASS               # 80 chunks per pass
            PCAP = PC * 128                 # 10240 idx per pass
            with ExitStack() as ph:
                sbe = ph.enter_context(tc.tile_pool(name="sbe", bufs=1))
                ACCT = sbe.tile([128, DBLK * 132], F32)
                hsI_t = sbe.tile([128, NCH], dt.int32)
                nc.sync.dma_start(hsI_t[:], hsI[:])
                adI_t = sbe.tile([128, NCH], dt.int32)
                nc.sync.dma_start(adI_t[:], adI[:])
                ea_t = sbe.tile([128, NCH], F32)
                nc.sync.dma_start(ea_t[:], eaC[:])
                df_t = sbe.tile([128, NCH], F32)
                nc.sync.dma_start(df_t[:], dstF[:])
                sm_t = sbe.tile([128, NCH], F32)
                nc.sync.dma_start(sm_t[:], selfM[:])
                for p in range(NPASS):
                    with ExitStack() as pp:
                        sbp = pp.enter_context(tc.tile_pool(name=f"sbp{p}", bufs=1))
                        sbo = pp.enter_context(tc.tile_pool(name=f"sbo{p}", bufs=4))
                        pse = pp.enter_context(tc.tile_pool(name=f"pse{p}", bufs=4,
                                                            space="PSUM"))
                        c0 = p * PC
                        NH = sbp.tile([128, PC * 192], F32, tag="NH")
                        AD = sbp.tile([128, PC * 64], F32, tag="AD")
                        SCT = sbp.tile([128, PC * 132], F32, tag="SCT")
                        S4 = sbp.tile([128, PC * 4], F32, tag="S4")
                        NHv = NH[:].rearrange("p (e w) -> p e w", w=192)
                        ADv = AD[:].rearrange("p (e w) -> p e w", w=64)
                        SCv = SCT[:].rearrange("p (e w) -> p e w", w=132)
                        S4v = S4[:].rearrange("p (e w) -> p e w", w=4)
                        for cc in range(PC):
                            nc.gpsimd.indirect_dma_start(
                                out=NHv[:, cc, :], out_offset=None, in_=h_tbl[:],
                                in_offset=bass.IndirectOffsetOnAxis(
                                    ap=hsI_t[:, c0 + cc:c0 + cc + 1], axis=0))
                            nc.gpsimd.indirect_dma_start(
                                out=ADv[:, cc, :], out_offset=None, in_=ad_tbl[:],
                                in_offset=bass.IndirectOffsetOnAxis(
                                    ap=adI_t[:, c0 + cc:c0 + cc + 1], axis=0))
                        # ea' = ea + selfM*mean ; q = a_src+a_dst+ea'*kap
                        EAm = sbp.tile([128, PC], F32, tag="EAm")
                        nc.vector.scalar_tensor_tensor(
                            out=EAm[:], in0=sm_t[:, c0:c0 + PC], scalar=meanr[:],
                            op0=ALU.mult, op1=ALU.add, in1=ea_t[:, c0:c0 + PC])
                        Q = sbp.tile([128, PC * 4], F32, tag="Q")
                        Qv = Q[:].rearrange("p (e w) -> p e w", w=4)
                        nc.vector.tensor_tensor(out=Qv, in0=NHv[:, :, 128:132],
                                                in1=ADv[:, :, 0:4], op=ALU.add)
                        T2 = sbp.tile([128, PC * 4], F32, tag="T2")
                        T2v = T2[:].rearrange("p (e w) -> p e w", w=4)
                        ea3 = EAm[:].rearrange("p (e w) -> p e w", w=1) \
                            .to_broadcast([128, PC, 4])
                        kap3 = kap_t[:].rearrange("p (o w) -> p o w", o=1) \
                            .to_broadcast([128, PC, 4])
                        nc.vector.tensor_tensor(out=T2v, in0=ea3, in1=kap3, op=ALU.mult)
                        nc.vector.tensor_tensor(out=Qv, in0=Qv, in1=T2v, op=ALU.add)
                        nc.vector.tensor_scalar_mul(T2v, Qv, LEAK)
                        nc.vector.tensor_tensor(out=Qv, in0=Qv, in1=T2v, op=ALU.max)
                        # exp(q) = sigmoid(q)/sigmoid(-q); Exp's ACT table
                        # is not resident (measured ~100x slowdown)
                        SG1 = sbp.tile([128, PC * 4], F32, tag="SG1")
                        nc.scalar.activation(SG1[:], Q[:], AF.Sigmoid)
                        nc.scalar.activation(S4[:], Q[:], AF.Sigmoid, scale=-1.0)
                        nc.vector.reciprocal(S4[:], S4[:])
                        nc.vector.tensor_tensor(out=S4[:], in0=SG1[:], in1=S4[:],
                                                op=ALU.mult)
                        nc.vector.tensor_copy(SCv[:, :, 128:132], S4v)
                        nh4 = NHv[:, :, 0:128].rearrange("p e (h c) -> p e h c", h=HEADS)
                        sc4 = S4v.rearrange("p e (h c) -> p e h c", c=1) \
                            .to_broadcast([128, PC, HEADS, C])
                        out4 = SCv[:, :, 0:128].rearrange("p e (h c) -> p e h c", h=HEADS)
                        nc.vector.tensor_tensor(out=out4, in0=nh4, in1=sc4, op=ALU.mult)
                        # one-hot binning into per-bucket PSUM accumulators
                        for c80 in range(PC):
                            cg = c0 + c80
                            bkt = cg // 2
                            first = (cg % 2 == 0)
                            last = (cg % 2 == 1)
                            oh = sbo.tile([128, 128], F32, tag="oh")
                            nc.vector.tensor_tensor(
                                out=oh[:],
                                in0=df_t[:, cg:cg + 1].to_broadcast([128, 128]),
                                in1=iota_t[:], op=ALU.is_equal)
                            if first:
                                pacc = pse.tile([128, 132], F32, space="PSUM", tag="pacc")
                            nc.tensor.matmul(pacc[:], lhsT=oh[:], rhs=SCv[:, c80, :],
                                             start=first, stop=last)
                            if last:
                                nc.vector.tensor_copy(
                                    ACCT[:, bkt * 132:(bkt + 1) * 132], pacc[:])
                nc.sync.dma_start(acc_tbl[:], ACCT[:])

        if STAGE >= 3:
            # ---------- A6: ReduceScatter ----------
            nc.gpsimd.collective_compute("ReduceScatter", ALU.add, replica_groups=RG,
                                         ins=[acc_tbl[:]], outs=[rs_out[:]])

        if STAGE >= 3:
            # ---------- A7: normalize + ReLU + transpose my slice ----------
            with ExitStack() as ph:
                sbn = ph.enter_context(tc.tile_pool(name="sbn", bufs=1))
                sbw = ph.enter_context(tc.tile_pool(name="sbw", bufs=4))
                psn = ph.enter_context(tc.tile_pool(name="psn", bufs=4, space="PSUM"))
                RSS = sbn.tile([16, DBLK * 132], F32)
                nc.sync.dma_start(RSS[:], rs_out[:])
                RSv = RSS[:].rearrange("p (j w) -> p j w", w=132)
                nc.vector.tensor_scalar_add(RSv[:, :, 128:132], RSv[:, :, 128:132], 1e-16)
                RC = sbn.tile([16, DBLK * 4], F32)
                RCv = RC[:].rearrange("p (j w) -> p j w", w=4)
                nc.vector.reciprocal(RCv, RSv[:, :, 128:132])
                r4 = RCv.rearrange("p j (h c) -> p j h c", c=1) \
                    .to_broadcast([16, DBLK, HEADS, C])
                m4 = RSv[:, :, 0:128].rearrange("p j (h c) -> p j h c", h=HEADS)
                nc.vector.tensor_tensor(out=m4, in0=m4, in1=r4, op=ALU.mult)
                gbb = gb_t[0:16, :].rearrange("p (o w) -> p o w", o=1) \
                    .to_broadcast([16, DBLK, 128])
                nc.vector.tensor_tensor(out=RSv[:, :, 0:128], in0=RSv[:, :, 0:128],
                                        in1=gbb, op=ALU.add)
                nc.vector.tensor_scalar_max(RSv[:, :, 0:128], RSv[:, :, 0:128], 0.0)
                GB = sbn.tile([128, DSL], F32)
                for j in range(DBLK):
                    ptr = psn.tile([128, 16], F32, space="PSUM", tag="ptr")
                    nc.tensor.transpose(out=ptr[:], in_=RSv[:, j, 0:128],
                                        identity=ident[0:16, 0:16])
                    nc.vector.tensor_copy(GB[:, j * 16:(j + 1) * 16], ptr[:])
                nc.sync.dma_start(gat_blob[:], GB[:])
        if debug:
            nc.sync.dma_start(dbg_gat[:], gat_blob[:])

        if STAGE >= 4:
            # ---------- A8: AllGather ----------
            nc.gpsimd.collective_compute("AllGather", ALU.bypass, replica_groups=RG,
                                         ins=[gat_blob[:]], outs=[gat_full[:]])

            # ---------- A9: gx = W_ih @ gat + b ----------
        persist = top.enter_context(tc.tile_pool(name="persist", bufs=1))
        gx = persist.tile([128, NT], F32)
        H = persist.tile([HID, NT + 32], F32)
        nc.gpsimd.memset(H[:], 0.0)
        if STAGE < 5:
            nc.gpsimd.memset(gx[:], 0.0)
        if STAGE >= 5:
            with ExitStack() as ph:
                sbg = ph.enter_context(tc.tile_pool(name="sbg", bufs=1))
                psg = ph.enter_context(tc.tile_pool(name="psg", bufs=4, space="PSUM"))
                GT = sbg.tile([128, NT], F32)
                GTv = GT[:].rearrange("p (j b r) -> p j b r", b=NCORES, r=16)
                for kk in range(NCORES):
                    nc.sync.dma_start(
                        GTv[:, :, kk, :],
                        gat_full[kk].rearrange("p (j r) -> p j r", r=16))
                for c in range(NT // 512):
                    pg = psg.tile([128, 512], F32, space="PSUM", tag="pg")
                    nc.tensor.matmul(pg[:], lhsT=wih_t[:], rhs=GT[:, c * 512:(c + 1) * 512],
                                     start=True, stop=True)
                    nc.vector.tensor_scalar_add(gx[:, c * 512:(c + 1) * 512], pg[:], br_t[:])
        if debug:
            nc.sync.dma_start(dbg_gx[:], gx[:])

        if STAGE >= 6:
            # ---------- A10: LSTM fixed point ----------
            with ExitStack() as ph:
                sbl = ph.enter_context(tc.tile_pool(name="sbl", bufs=2))
                sbl3 = ph.enter_context(tc.tile_pool(name="sbl3", bufs=3))
                psl = ph.enter_context(tc.tile_pool(name="psl", bufs=2, space="PSUM"))
                for it in range(ITERS):
                    Cprev = None
                    for s in range(NSC):
                        lo, hi = s * SC, (s + 1) * SC
                        if it == 0:
                            Gp = gx[:, lo:hi]
                        else:
                            pG = psl.tile([128, SC], F32, space="PSUM", tag="pG")
                            for q in range(SC // 512):
                                nc.tensor.matmul(pG[:, q * 512:(q + 1) * 512], lhsT=whh_t[:],
                                                 rhs=H[:, lo + q * 512:lo + (q + 1) * 512],
                                                 start=True, stop=True)
                            Gs = sbl.tile([128, SC], F32, tag="Gs")
                            nc.vector.tensor_tensor(out=Gs[:], in0=pG[:], in1=gx[:, lo:hi],
                                                    op=ALU.add)
                            Gp = Gs[:]
                        S_ = sbl.tile([96, SC], F32, tag="S")
                        nc.scalar.activation(S_[:], Gp[0:96, :], AF.Sigmoid)
                        Tg = sbl.tile([64, SC], F32, tag="Tg")
                        nc.scalar.activation(Tg[32:64, :], Gp[96:128, :], AF.Tanh)
                        Zt = sbl3.tile([HID, SC], F32, tag="Zt")
                        nc.vector.tensor_tensor(out=Zt[:], in0=S_[32:64, :],
                                                in1=Tg[32:64, :], op=ALU.mult)
                        Ct = sbl3.tile([HID, SC], F32, tag="Ct")
                        nc.vector.tensor_tensor_scan(
                            out=Ct[:], data0=S_[0:32, :], data1=Zt[:],
                            initial=(0.0 if Cprev is None else Cprev[:, SC - 1:SC]),
                            op0=ALU.mult, op1=ALU.add)
                        TC = sbl.tile([96, SC], F32, tag="TC")
                        nc.scalar.activation(TC[64:96, :], Ct[:], AF.Tanh)
                        nc.vector.tensor_tensor(out=H[:, lo + 1:hi + 1], in0=S_[64:96, :],
                                                in1=TC[64:96, :], op=ALU.mult)
                        Cprev = Ct
        if debug:
            nc.sync.dma_start(dbg_h[:], H[:, 1:NT + 1])

        if STAGE >= 7:
            # ---------- FC ----------
            with ExitStack() as ph:
                sbf = ph.enter_context(tc.tile_pool(name="sbf", bufs=1))
                psf = ph.enter_context(tc.tile_pool(name="psf", bufs=4, space="PSUM"))
                OF = sbf.tile([1, NT], F32)
                for c in range(NT // 512):
                    pf = psf.tile([1, 512], F32, space="PSUM", tag="pf")
                    nc.tensor.matmul(pf[:], lhsT=wfc_t[:],
                                     rhs=H[:, 1 + c * 512:1 + (c + 1) * 512],
                                     start=True, stop=True)
                    nc.vector.tensor_scalar_add(OF[:, c * 512:(c + 1) * 512], pf[:], bfc_t[:])
                nc.sync.dma_start(out[:], OF[:])

    nc.compile()
    return nc


def run(inputs, trace=False, debug=False):
    key = ("dbg" if debug else "rel")
    if key not in _CACHE:
        _CACHE[key] = _build_nc(debug=debug)
    nc = _CACHE[key]
    in_maps = _prep_host(inputs)
    res = run_bass_kernel_spmd(nc, in_maps, list(range(NCORES)), trace=trace)
    return res


def kernel(**inputs) -> np.ndarray:
    res = run(inputs)
    o = res.results[0]["out"]
    return np.ascontiguousarray(o[0, :N].reshape(N, 1).astype(np.float32))



# revision 8
# speedup vs baseline: 5.5079x; 5.5079x over previous
"""GAT+LSTM fused kernel for 8 trn2 NeuronCores (v2).

Key structure (per core, fully collective-free):
- Output depends only on batch row T-1=11 of the reference LSTM, so only
  GAT outputs for live nodes [110000, 120000) are needed.
- Live nodes split into 80 buckets of 128 by dst>>7. Core k owns buckets
  [10k-1, 10k+10): its 1280 output nodes PLUS the 128-node bucket that
  contains its 96-step LSTM warmup window (recomputed redundantly, so no
  cross-core exchange is needed anywhere).
- Edges partitioned by dst bucket. Per 128-edge chunk: indirect-gather
  x[src] rows (bf16), PE-transpose pairs, h|a_src via one bf16 matmul,
  a_dst via one-hot transpose matmul, segment softmax without max
  subtraction, one-hot scatter matmul accumulating [dst,132] in PSUM.
- Self-loops handled densely from the core's own node block (no gather);
  their edge_attr is mean(edge_attr), reduced locally from the full
  edge_attr (replicated input) - no AllReduce.
- LSTM: 4 chunks of 320 steps + 96-step warmup packed on partitions,
  fixed-point iterations (ITERS=5 converges to ~2e-4 in fp32, ~4e-3 with
  bf16 data paths; tolerance is 2e-2). Elementwise work split across
  vector/gpsimd/scalar engines.
- FC on the 4x320 main columns; host concatenates the 8 per-core slices.
"""
import os
import numpy as np
import ml_dtypes

import concourse.bass as bass
import concourse.bacc as bacc
import concourse.tile as tile
from concourse import mybir
from concourse.bass_utils import run_bass_kernel_spmd
from concourse.masks import make_identity
from contextlib import ExitStack

dt = mybir.dt
F32 = dt.float32
BF16 = dt.bfloat16
AF = mybir.ActivationFunctionType
ALU = mybir.AluOpType

T, N, F_IN = 12, 10000, 64
HEADS, C, HID = 4, 32, 32
E, TN = 1_000_000, 120_000
NCORES = 8
D0 = (T - 1) * N
DN = N
NB = 11                      # buckets per core (1 halo + 10 own)
NW = 96                      # LSTM warmup steps
LG = 320                     # LSTM group length
NGRP = 4
LGW = LG + NW                # 416
NTL = NGRP * LGW             # 1664
SEQ = NGRP * LG              # 1280 sequence cols owned per core
ITERS = 5
LEAK = 0.2
XPAD = TN + 64               # x table rows (pad rows are zero)
EAC = (E + 127) // 128       # 7813 cols for the local edge_attr reduce

_CACHE = {}


def _prep_host(inputs):
    x = np.asarray(inputs["x_seq"], np.float32).reshape(TN, F_IN)
    ei = np.asarray(inputs["edge_index"])
    ea = np.asarray(inputs["edge_attr"], np.float32)[:, 0]
    W_gat = np.asarray(inputs["W_gat"], np.float32)
    att_src = np.asarray(inputs["att_src"], np.float32)
    att_dst = np.asarray(inputs["att_dst"], np.float32)
    att_edge = np.asarray(inputs["att_edge"], np.float32)
    W_edge = np.asarray(inputs["W_edge"], np.float32)
    gat_bias = np.asarray(inputs["gat_bias"], np.float32)
    W_ih = np.asarray(inputs["W_ih"], np.float32)
    W_hh = np.asarray(inputs["W_hh"], np.float32)
    b = np.asarray(inputs["b_ih"], np.float32) + np.asarray(inputs["b_hh"], np.float32)
    W_fc = np.asarray(inputs["W_fc"], np.float32)
    b_fc = np.asarray(inputs["b_fc"], np.float32)

    # W_all: [64, 136] = [W_gat | W_gat@A_src | W_gat@A_dst]
    A_src = np.zeros((HEADS * C, HEADS), np.float32)
    A_dst = np.zeros((HEADS * C, HEADS), np.float32)
    for h in range(HEADS):
        A_src[h * C:(h + 1) * C, h] = att_src[h]
        A_dst[h * C:(h + 1) * C, h] = att_dst[h]
    W_all = np.concatenate([W_gat, W_gat @ A_src, W_gat @ A_dst], axis=1)
    kap = np.array([np.dot(W_edge[0, h * C:(h + 1) * C], att_edge[h])
                    for h in range(HEADS)], np.float32)
    kap_rep = np.broadcast_to(kap, (128, HEADS)).copy()
    gb_row = np.broadcast_to(gat_bias, (128, HEADS * C)).copy()
    iota128 = np.broadcast_to(np.arange(128, dtype=np.float32), (128, 128)).copy()
    # gate row order [f, i, o, g] (torch order is i,f,g,o)
    perm = np.concatenate([np.arange(32, 64), np.arange(0, 32),
                           np.arange(96, 128), np.arange(64, 96)])
    WihT = np.ascontiguousarray(W_ih[perm].T)
    WhhT = np.ascontiguousarray(W_hh[perm].T)
    br = np.ascontiguousarray(b[perm].reshape(128, 1))

    xbf = np.zeros((XPAD, F_IN), ml_dtypes.bfloat16)
    xbf[:TN] = x
    eaF = np.zeros((128, EAC), np.float32)
    j = np.arange(E)
    eaF[j % 128, j // 128] = ea

    src = ei[0].astype(np.int64)
    dst = ei[1].astype(np.int64)
    live = (dst >= D0) & (dst < D0 + DN)
    sl = src[live]
    dl = dst[live] - D0
    bkt = dl >> 7
    cnt = np.bincount(bkt, minlength=80)
    chb = int(max(1, -(-int(cnt.max()) // 128)))   # chunks per bucket
    nch = NB * chb

    by_bucket = {}
    order = np.argsort(bkt, kind="stable")
    pos = 0
    for gb in range(80):
        by_bucket[gb] = order[pos:pos + cnt[gb]]
        pos += cnt[gb]

    in_maps = []
    for k in range(NCORES):
        xI = np.full((128, nch), TN, np.int32)
        eaC = np.zeros((128, nch), np.float32)
        dstF = np.full((128, nch), -1.0, np.float32)
        for b_ in range(NB):
            gb = 10 * k - 1 + b_
            if not (0 <= gb < 80):
                continue
            sel = by_bucket[gb]
            ne = len(sel)
            assert ne <= chb * 128
            ii = np.arange(ne)
            cc = b_ * chb + ii // 128
            pp = ii % 128
            xI[pp, cc] = sl[sel].astype(np.int32)
            eaC[pp, cc] = ea[live][sel]
            dstF[pp, cc] = (dl[sel] - 128 * gb).astype(np.float32)
        # own node features, transposed: bucket-local nodes [128*(10k-1), +1408)
        xTD = np.zeros((F_IN, NB * 128), ml_dtypes.bfloat16)
        lo = 128 * (10 * k - 1)
        for j2 in range(NB * 128):
            gn = lo + j2
            if 0 <= gn < DN:
                xTD[:, j2] = x[D0 + gn]
        Bwarm = np.zeros((128, NW), np.float32)
        if k == 0:
            Bwarm[32:96, :] = -30.0    # i and o gate rows of the junk warmup
        in_maps.append({
            "xnd": xbf, "eaF": eaF,
            "xI": xI, "eaC": eaC, "dstF": dstF,
            "xTD": np.ascontiguousarray(xTD),
            "Bwarm": Bwarm,
            "Wall": W_all.astype(ml_dtypes.bfloat16),
            "kap": kap_rep, "gbrow": gb_row, "iota": iota128,
            "Wih": WihT.astype(ml_dtypes.bfloat16),
            "Whh": WhhT.astype(ml_dtypes.bfloat16),
            "br": br,
            "Wfc": np.ascontiguousarray(W_fc.reshape(HID, 1)).astype(ml_dtypes.bfloat16),
            "bfc": np.ascontiguousarray(b_fc.reshape(1, 1)),
        })
    return in_maps, chb


def _build_nc(chb):
    nch = NB * chb
    nc = bacc.Bacc("TRN2", target_bir_lowering=False, debug=False,
                   num_devices=NCORES)
    g = lambda n, s, d=F32: nc.dram_tensor(n, s, d, kind="ExternalInput").ap()
    xnd = g("xnd", [XPAD, F_IN], BF16)
    eaF = g("eaF", [128, EAC])
    xI = g("xI", [128, nch], dt.int32)
    eaC = g("eaC", [128, nch])
    dstF = g("dstF", [128, nch])
    xTD = g("xTD", [F_IN, NB * 128], BF16)
    Bwarm = g("Bwarm", [128, NW])
    Wall = g("Wall", [F_IN, 136], BF16)
    kap = g("kap", [128, HEADS])
    gbrow = g("gbrow", [128, 128])
    iota = g("iota", [128, 128])
    Wih = g("Wih", [128, 128], BF16)
    Whh = g("Whh", [HID, 128], BF16)
    br = g("br", [128, 1])
    Wfc = g("Wfc", [HID, 1], BF16)
    bfc = g("bfc", [1, 1])
    out = nc.dram_tensor("out", [1, SEQ], F32, kind="ExternalOutput").ap()

    with tile.TileContext(nc) as tc, ExitStack() as top:
        const = top.enter_context(tc.tile_pool(name="const", bufs=1))
        identB = const.tile([128, 128], BF16)
        make_identity(nc, identB[:])
        wall_t = const.tile([F_IN, 136], BF16); nc.sync.dma_start(wall_t[:], Wall[:])
        kap_t = const.tile([128, HEADS], F32); nc.sync.dma_start(kap_t[:], kap[:])
        gbr_t = const.tile([128, 128], F32); nc.sync.dma_start(gbr_t[:], gbrow[:])
        iota_t = const.tile([128, 128], F32); nc.sync.dma_start(iota_t[:], iota[:])
        wih_t = const.tile([128, 128], BF16); nc.sync.dma_start(wih_t[:], Wih[:])
        whh_t = const.tile([HID, 128], BF16); nc.sync.dma_start(whh_t[:], Whh[:])
        br_t = const.tile([128, 1], F32); nc.sync.dma_start(br_t[:], br[:])
        wfc_t = const.tile([HID, 1], BF16); nc.sync.dma_start(wfc_t[:], Wfc[:])
        bfc_t = const.tile([1, 1], F32); nc.sync.dma_start(bfc_t[:], bfc[:])
        bw_t = const.tile([128, NW], F32); nc.sync.dma_start(bw_t[:], Bwarm[:])
        xi_t = const.tile([128, nch], dt.int32); nc.sync.dma_start(xi_t[:], xI[:])
        eac_t = const.tile([128, nch], F32); nc.sync.dma_start(eac_t[:], eaC[:])
        dsf_t = const.tile([128, nch], F32); nc.sync.dma_start(dsf_t[:], dstF[:])
        xtd_t = const.tile([F_IN, NB * 128], BF16); nc.sync.dma_start(xtd_t[:], xTD[:])
        meanr = const.tile([128, 1], F32)
        gatT = const.tile([128, NB * 128], BF16)     # [feat, bucket-local node]

        # ---------- Phase 0: mean(edge_attr), local full reduce ----------
        with ExitStack() as ph:
            sbm = ph.enter_context(tc.tile_pool(name="sbm", bufs=1))
            psm = ph.enter_context(tc.tile_pool(name="psm", bufs=1, space="PSUM"))
            eaf_t = sbm.tile([128, EAC], F32)
            nc.sync.dma_start(eaf_t[:], eaF[:])
            eap = sbm.tile([128, 1], F32)
            nc.vector.tensor_reduce(eap[:], eaf_t[:], mybir.AxisListType.X, ALU.add)
            onc = sbm.tile([128, 1], F32)
            nc.gpsimd.memset(onc[:], 1.0)
            ps1 = psm.tile([1, 1], F32, space="PSUM", tag="ps1")
            nc.tensor.matmul(ps1[:], lhsT=eap[:], rhs=onc[:], start=True, stop=True)
            eas = sbm.tile([1, 1], F32)
            nc.scalar.mul(eas[:], ps1[:], 1.0 / E)
            onr = sbm.tile([1, 128], F32)
            nc.gpsimd.memset(onr[:], 1.0)
            ps2 = psm.tile([128, 1], F32, space="PSUM", tag="ps2")
            nc.tensor.matmul(ps2[:], lhsT=onr[:], rhs=eas[:], start=True, stop=True)
            nc.vector.tensor_copy(meanr[:], ps2[:])

        # ---------- Phase 1: self tables (h|a_src|a_dst for own nodes) ----
        sfp = top.enter_context(tc.tile_pool(name="sfp", bufs=1))
        SF = sfp.tile([128, NB * 136], F32)
        SFv = SF[:].rearrange("p (j w) -> p j w", w=136)
        adB = sfp.tile([128, NB * 4], BF16)
        adBv = adB[:].rearrange("p (j w) -> p j w", w=4)
        selfSC = sfp.tile([128, NB * 132], BF16)
        sSCv = selfSC[:].rearrange("p (j w) -> p j w", w=132)
        with ExitStack() as ph:
            sbs = ph.enter_context(tc.tile_pool(name="sbs", bufs=1))
            pss = ph.enter_context(tc.tile_pool(name="pss", bufs=4, space="PSUM"))
            for b_ in range(NB):
                pf = pss.tile([128, 136], F32, space="PSUM", tag="pf")
                nc.tensor.matmul(pf[:], lhsT=xtd_t[:, b_ * 128:(b_ + 1) * 128],
                                 rhs=wall_t[:], start=True, stop=True)
                nc.vector.tensor_copy(SFv[:, b_, :], pf[:])
            nc.vector.tensor_copy(adBv, SFv[:, :, 132:136])
            QS = sbs.tile([128, NB * 4], F32)
            QSv = QS[:].rearrange("p (j w) -> p j w", w=4)
            nc.vector.tensor_tensor(out=QSv, in0=SFv[:, :, 128:132],
                                    in1=SFv[:, :, 132:136], op=ALU.add)
            kapb = kap_t[:].rearrange("p (o w) -> p o w", o=1) \
                .to_broadcast([128, NB, 4])
            nc.vector.scalar_tensor_tensor(out=QSv, in0=kapb, scalar=meanr[:],
                                           op0=ALU.mult, op1=ALU.add, in1=QSv)
            T2 = sbs.tile([128, NB * 4], F32)
            nc.vector.tensor_scalar_mul(T2[:], QS[:], LEAK)
            nc.vector.tensor_tensor(out=QS[:], in0=QS[:], in1=T2[:], op=ALU.max)
            SG1 = sbs.tile([128, NB * 4], F32)
            nc.scalar.activation(SG1[:], QS[:], AF.Sigmoid)
            nc.scalar.activation(T2[:], QS[:], AF.Sigmoid, scale=-1.0)
            nc.vector.reciprocal(T2[:], T2[:])
            SSf = sbs.tile([128, NB * 4], F32)
            nc.vector.tensor_tensor(out=SSf[:], in0=SG1[:], in1=T2[:], op=ALU.mult)
            ssf4 = SSf[:].rearrange("p (j h w) -> p j h w", h=HEADS, w=1) \
                .to_broadcast([128, NB, HEADS, C])
            sf4 = SFv[:, :, 0:128].rearrange("p j (h c) -> p j h c", h=HEADS)
            o4 = sSCv[:, :, 0:128].rearrange("p j (h c) -> p j h c", h=HEADS)
            nc.vector.tensor_tensor(out=o4, in0=sf4, in1=ssf4, op=ALU.mult)
            nc.vector.tensor_copy(sSCv[:, :, 128:132],
                                  SSf[:].rearrange("p (j w) -> p j w", w=4))

        # ---------- Phase 2: edge phase ----------
        NXT = (chb + 1) // 2          # transpose pair count per bucket
        with ExitStack() as ph:
            sbe = ph.enter_context(tc.tile_pool(name="sbe", bufs=2))
            sbq = ph.enter_context(tc.tile_pool(name="sbq", bufs=2))
            pse = ph.enter_context(tc.tile_pool(name="pse", bufs=2, space="PSUM"))
            psh = ph.enter_context(tc.tile_pool(name="psh", bufs=2, space="PSUM"))
            psa = ph.enter_context(tc.tile_pool(name="psa", bufs=1, space="PSUM"))
            psk = ph.enter_context(tc.tile_pool(name="psk", bufs=2, space="PSUM"))
            for b_ in range(NB):
                c0 = b_ * chb
                XGB = sbe.tile([128, chb * 64], BF16, tag="XGB")
                XGBv = XGB[:].rearrange("p (e w) -> p e w", w=64)
                for cc in range(chb):
                    nc.gpsimd.indirect_dma_start(
                        out=XGBv[:, cc, :], out_offset=None, in_=xnd[:],
                        in_offset=bass.IndirectOffsetOnAxis(
                            ap=xi_t[:, c0 + cc:c0 + cc + 1], axis=0))
                XT = sbe.tile([F_IN, chb * 128], BF16, tag="XT")
                for pr in range(NXT):
                    w = min(128, chb * 64 - pr * 128)
                    pxT = psk.tile([128, 128], BF16, space="PSUM", tag="ptr")
                    nc.tensor.transpose(out=pxT[0:w, :],
                                        in_=XGB[:, pr * 128:pr * 128 + w],
                                        identity=identB[:])
                    nc.vector.tensor_copy(XT[:, (2 * pr) * 128:(2 * pr + 1) * 128],
                                          pxT[0:64, :])
                    if w > 64:
                        nc.vector.tensor_copy(
                            XT[:, (2 * pr + 1) * 128:(2 * pr + 2) * 128],
                            pxT[64:128, :])
                # h|a_src per edge; batches of 3 chunks share one PSUM tile
                hEs = sbe.tile([128, chb * 132], BF16, tag="hEs")
                hEv = hEs[:].rearrange("p (e w) -> p e w", w=132)
                Qb = sbq.tile([128, chb * 4], F32, tag="Qb")
                Qv = Qb[:].rearrange("p (e w) -> p e w", w=4)
                nb3 = (chb + 2) // 3
                for b3 in range(nb3):
                    n3 = min(3, chb - b3 * 3)
                    phE = psh.tile([128, 3 * 132], F32, space="PSUM", tag="phE")
                    for j3 in range(n3):
                        cc = b3 * 3 + j3
                        lhs = XT[:, cc * 128:(cc + 1) * 128]
                        nc.tensor.matmul(phE[:, j3 * 132:(j3 + 1) * 132],
                                         lhsT=lhs, rhs=wall_t[:, 0:132],
                                         start=True, stop=True)
                    phEv = phE[:].rearrange("p (e w) -> p e w", w=132)
                    nc.vector.tensor_copy(hEv[:, b3 * 3:b3 * 3 + n3, :],
                                          phEv[:, 0:n3, :])
                    nc.vector.tensor_copy(Qv[:, b3 * 3:b3 * 3 + n3, :],
                                          phEv[:, 0:n3, 128:132])
                # one-hots for the whole bucket in one op
                ohs = sbe.tile([128, chb * 128], BF16, tag="ohs")
                ohv = ohs[:].rearrange("p (e w) -> p e w", w=128)
                dfb = dsf_t[:, c0:c0 + chb].rearrange("p (e w) -> p e w", w=1) \
                    .to_broadcast([128, chb, 128])
                iob = iota_t[:].rearrange("p (o w) -> p o w", o=1) \
                    .to_broadcast([128, chb, 128])
                nc.vector.tensor_tensor(out=ohv, in0=dfb, in1=iob, op=ALU.is_equal)
                # a_dst per edge via transposed one-hot
                padc = psa.tile([128, chb * 4], F32, space="PSUM", tag="padc")
                for cc in range(chb):
                    pohT = psk.tile([128, 128], BF16, space="PSUM", tag="ptr")
                    nc.tensor.transpose(out=pohT[:], in_=ohv[:, cc, :],
                                        identity=identB[:])
                    ohTs = sbq.tile([128, 128], BF16, tag="ohTs")
                    nc.vector.tensor_copy(ohTs[:], pohT[:])
                    nc.tensor.matmul(padc[:, cc * 4:(cc + 1) * 4], lhsT=ohTs[:],
                                     rhs=adBv[:, b_, :], start=True, stop=True)
                # q = a_src + a_dst + ea*kap ; s = exp(leaky_relu(q))
                kmb = sbq.tile([128, chb * 4], F32, tag="kmb")
                kmv = kmb[:].rearrange("p (e w) -> p e w", w=4)
                eab = eac_t[:, c0:c0 + chb].rearrange("p (e w) -> p e w", w=1) \
                    .to_broadcast([128, chb, 4])
                kab = kap_t[:].rearrange("p (o w) -> p o w", o=1) \
                    .to_broadcast([128, chb, 4])
                nc.vector.tensor_tensor(out=kmv, in0=eab, in1=kab, op=ALU.mult)
                nc.vector.tensor_tensor(out=Qb[:], in0=Qb[:], in1=kmb[:], op=ALU.add)
                nc.vector.tensor_tensor(out=Qb[:], in0=Qb[:], in1=padc[:], op=ALU.add)
                nc.vector.tensor_scalar_mul(kmb[:], Qb[:], LEAK)
                nc.vector.tensor_tensor(out=Qb[:], in0=Qb[:], in1=kmb[:], op=ALU.max)
                SG = sbq.tile([128, chb * 4], F32, tag="SG")
                nc.scalar.activation(SG[:], Qb[:], AF.Sigmoid)
                nc.scalar.activation(kmb[:], Qb[:], AF.Sigmoid, scale=-1.0)
                nc.vector.reciprocal(kmb[:], kmb[:])
                Sbf = sbq.tile([128, chb * 4], BF16, tag="Sbf")
                nc.vector.tensor_tensor(out=Sbf[:], in0=SG[:], in1=kmb[:], op=ALU.mult)
                Sbv = Sbf[:].rearrange("p (e w) -> p e w", w=4)
                # messages and scatter
                SCb = sbe.tile([128, chb * 132], BF16, tag="SCb")
                SCv = SCb[:].rearrange("p (e w) -> p e w", w=132)
                sb4 = Sbf[:].rearrange("p (e h w) -> p e h w", h=HEADS, w=1) \
                    .to_broadcast([128, chb, HEADS, C])
                he4 = hEv[:, :, 0:128].rearrange("p e (h c) -> p e h c", h=HEADS)
                sc4 = SCv[:, :, 0:128].rearrange("p e (h c) -> p e h c", h=HEADS)
                nc.vector.tensor_tensor(out=sc4, in0=he4, in1=sb4, op=ALU.mult)
                nc.vector.tensor_copy(SCv[:, :, 128:132], Sbv)
                pacc = pse.tile([128, 132], F32, space="PSUM", tag="pacc")
                for cc in range(chb):
                    nc.tensor.matmul(pacc[:], lhsT=ohv[:, cc, :], rhs=SCv[:, cc, :],
                                     start=(cc == 0), stop=(cc == chb - 1))
                # add self loops, normalize, bias, relu, transpose
                nc.vector.tensor_tensor(out=pacc[:], in0=pacc[:],
                                        in1=sSCv[:, b_, :], op=ALU.add)
                dn = sbq.tile([128, 4], F32, tag="dn")
                nc.vector.tensor_scalar_add(dn[:], pacc[:, 128:132], 1e-16)
                nc.vector.reciprocal(dn[:], dn[:])
                gn = sbq.tile([128, 128], F32, tag="gn")
                g4 = gn[:].rearrange("p (h c) -> p h c", h=HEADS)
                p4 = pacc[:, 0:128].rearrange("p (h c) -> p h c", h=HEADS)
                d4 = dn[:].rearrange("p (h w) -> p h w", w=1) \
                    .to_broadcast([128, HEADS, C])
                nc.vector.tensor_tensor(out=g4, in0=p4, in1=d4, op=ALU.mult)
                nc.vector.tensor_tensor(out=gn[:], in0=gn[:], in1=gbr_t[:], op=ALU.add)
                gnb = sbq.tile([128, 128], BF16, tag="gnb")
                nc.vector.tensor_scalar_max(gnb[:], gn[:], 0.0)
                pgT = psk.tile([128, 128], BF16, space="PSUM", tag="ptr")
                nc.tensor.transpose(out=pgT[:], in_=gnb[:], identity=identB[:])
                nc.vector.tensor_copy(gatT[:, b_ * 128:(b_ + 1) * 128], pgT[:])

        # ---------- Phase 3: gx = Wih.T @ gatT (+ warmup mask) ----------
        persist = top.enter_context(tc.tile_pool(name="persist", bufs=1))
        gx = persist.tile([128, NTL], F32)
        H = persist.tile([HID, NTL], BF16)
        nc.gpsimd.memset(H[:], 0.0)
        with ExitStack() as ph:
            psg = ph.enter_context(tc.tile_pool(name="psg", bufs=4, space="PSUM"))
            for g_ in range(NGRP):
                pgx = psg.tile([128, LGW], F32, space="PSUM", tag="pgx")
                nc.tensor.matmul(pgx[:], lhsT=wih_t[:],
                                 rhs=gatT[:, 320 * g_ + 32:320 * g_ + 448],
                                 start=True, stop=True)
                nc.vector.tensor_copy(gx[:, g_ * LGW:(g_ + 1) * LGW], pgx[:])
            nc.vector.tensor_tensor(out=gx[:, 0:NW], in0=gx[:, 0:NW],
                                    in1=bw_t[:], op=ALU.add)

        # ---------- Phase 4: LSTM fixed point ----------
        YS = persist.tile([HID, SEQ], BF16)
        with ExitStack() as ph:
            sbl = ph.enter_context(tc.tile_pool(name="sbl", bufs=2))
            psl = ph.enter_context(tc.tile_pool(name="psl", bufs=1, space="PSUM"))
            for it in range(ITERS):
                if it == 0:
                    Gp = gx[:]
                else:
                    pG = psl.tile([128, 2048], F32, space="PSUM", tag="pG")
                    for q in range(4):
                        lo = q * 512
                        hi = min(NTL, lo + 512)
                        nc.tensor.matmul(pG[:, lo:hi], lhsT=whh_t[:],
                                         rhs=H[:, lo:hi], start=True, stop=True)
                    Gs = sbl.tile([128, NTL], F32, tag="Gs")
                    nc.vector.tensor_tensor(out=Gs[:], in0=pG[:, 0:NTL],
                                            in1=gx[:], op=ALU.add)
                    Gp = Gs[:]
                S_ = sbl.tile([96, NTL], BF16, tag="S")
                nc.scalar.activation(S_[:], Gp[0:96, :], AF.Sigmoid,
                                     bias=br_t[0:96, :])
                Tg = sbl.tile([64, NTL], BF16, tag="Tg")
                nc.scalar.activation(Tg[32:64, :], Gp[96:128, :], AF.Tanh,
                                     bias=br_t[96:128, :])
                Zp = sbl.tile([128, LGW], BF16, tag="Zp")
                Fp = sbl.tile([128, LGW], BF16, tag="Fp")
                for g_ in range(NGRP):
                    sl_ = slice(g_ * LGW, (g_ + 1) * LGW)
                    nc.vector.tensor_tensor(out=Zp[g_ * 32:(g_ + 1) * 32, :],
                                            in0=S_[32:64, sl_], in1=Tg[32:64, sl_],
                                            op=ALU.mult)
                    nc.gpsimd.tensor_copy(Fp[g_ * 32:(g_ + 1) * 32, :],
                                          S_[0:32, sl_])
                Ct = sbl.tile([128, LGW], BF16, tag="Ct")
                nc.vector.tensor_tensor_scan(out=Ct[:], data0=Fp[:], data1=Zp[:],
                                             initial=0.0, op0=ALU.mult, op1=ALU.add)
                TCu = sbl.tile([96, NTL], BF16, tag="TCu")
                for g_ in range(NGRP):
                    nc.gpsimd.tensor_copy(
                        TCu[64:96, g_ * LGW:(g_ + 1) * LGW],
                        Ct[g_ * 32:(g_ + 1) * 32, :])
                nc.scalar.activation(TCu[64:96, :], TCu[64:96, :], AF.Tanh)
                if it < ITERS - 1:
                    for g_ in range(NGRP):
                        nc.gpsimd.tensor_tensor(
                            out=H[0:32, g_ * LGW + 1:(g_ + 1) * LGW],
                            in0=S_[64:96, g_ * LGW:(g_ + 1) * LGW - 1],
                            in1=TCu[64:96, g_ * LGW:(g_ + 1) * LGW - 1], op=ALU.mult)
                else:
                    for g_ in range(NGRP):
                        nc.vector.tensor_tensor(
                            out=YS[:, g_ * LG:(g_ + 1) * LG],
                            in0=S_[64:96, g_ * LGW + NW:(g_ + 1) * LGW],
                            in1=TCu[64:96, g_ * LGW + NW:(g_ + 1) * LGW], op=ALU.mult)

        # ---------- Phase 5: FC ----------
        with ExitStack() as ph:
            sbf = ph.enter_context(tc.tile_pool(name="sbf", bufs=1))
            psf = ph.enter_context(tc.tile_pool(name="psf", bufs=4, space="PSUM"))
            OF = sbf.tile([1, SEQ], F32)
            for g_ in range(NGRP):
                pf = psf.tile([1, LG], F32, space="PSUM", tag="pfc")
                nc.tensor.matmul(pf[:], lhsT=wfc_t[:],
                                 rhs=YS[:, g_ * LG:(g_ + 1) * LG],
                                 start=True, stop=True)
                nc.vector.tensor_scalar_add(OF[:, g_ * LG:(g_ + 1) * LG],
                                            pf[:], bfc_t[:])
            nc.sync.dma_start(out[:], OF[:])

    nc.compile()
    return nc


def run(inputs, trace=False):
    in_maps, chb = _prep_host(inputs)
    if chb not in _CACHE:
        _CACHE[chb] = _build_nc(chb)
    nc = _CACHE[chb]
    res = run_bass_kernel_spmd(nc, in_maps, list(range(NCORES)), trace=trace)
    return res


def kernel(**inputs) -> np.ndarray:
    res = run(inputs)
    full = np.concatenate([np.asarray(res.results[k]["out"][0], np.float32)
                           for k in range(NCORES)])
    return np.ascontiguousarray(full[:N].reshape(N, 1))


# revision 14
# speedup vs baseline: 7.7027x; 1.3985x over previous
"""GAT+LSTM fused kernel for 8 trn2 NeuronCores (v2).

Key structure (per core, fully collective-free):
- Output depends only on batch row T-1=11 of the reference LSTM, so only
  GAT outputs for live nodes [110000, 120000) are needed.
- Live nodes split into 80 buckets of 128 by dst>>7. Core k owns buckets
  [10k-1, 10k+10): its 1280 output nodes PLUS the 128-node bucket that
  contains its 96-step LSTM warmup window (recomputed redundantly, so no
  cross-core exchange is needed anywhere).
- Edges partitioned by dst bucket. Per 128-edge chunk: indirect-gather
  x[src] rows (bf16), PE-transpose pairs, h|a_src via one bf16 matmul,
  a_dst via one-hot transpose matmul, segment softmax without max
  subtraction, one-hot scatter matmul accumulating [dst,132] in PSUM.
- Self-loops handled densely from the core's own node block (no gather);
  their edge_attr is mean(edge_attr), reduced locally from the full
  edge_attr (replicated input) - no AllReduce.
- LSTM: 4 chunks of 320 steps + 96-step warmup packed on partitions,
  fixed-point iterations (ITERS=5 converges to ~2e-4 in fp32, ~4e-3 with
  bf16 data paths; tolerance is 2e-2). Elementwise work split across
  vector/gpsimd/scalar engines.
- FC on the 4x320 main columns; host concatenates the 8 per-core slices.
"""
import os
import numpy as np
import ml_dtypes

import concourse.bass as bass
import concourse.bacc as bacc
import concourse.tile as tile
from concourse import mybir
from concourse.bass_utils import run_bass_kernel_spmd
from concourse.masks import make_identity
from contextlib import ExitStack

dt = mybir.dt
F32 = dt.float32
BF16 = dt.bfloat16
AF = mybir.ActivationFunctionType
ALU = mybir.AluOpType

T, N, F_IN = 12, 10000, 64
HEADS, C, HID = 4, 32, 32
E, TN = 1_000_000, 120_000
NCORES = 8
D0 = (T - 1) * N
DN = N
NB = 11                      # buckets per core (1 halo + 10 own)
NW = 96                      # LSTM warmup steps
SEQ = 1280                   # sequence cols owned per core
NTL = SEQ + NW               # 1376: one chained sequence incl warmup
ITERS = 4
LEAK = 0.2
XPAD = TN + 64               # x table rows (pad rows are zero)
EAC = (E + 127) // 128       # 7813 cols for the local edge_attr reduce

_CACHE = {}


def _prep_host(inputs):
    x = np.asarray(inputs["x_seq"], np.float32).reshape(TN, F_IN)
    ei = np.asarray(inputs["edge_index"])
    ea = np.asarray(inputs["edge_attr"], np.float32)[:, 0]
    W_gat = np.asarray(inputs["W_gat"], np.float32)
    att_src = np.asarray(inputs["att_src"], np.float32)
    att_dst = np.asarray(inputs["att_dst"], np.float32)
    att_edge = np.asarray(inputs["att_edge"], np.float32)
    W_edge = np.asarray(inputs["W_edge"], np.float32)
    gat_bias = np.asarray(inputs["gat_bias"], np.float32)
    W_ih = np.asarray(inputs["W_ih"], np.float32)
    W_hh = np.asarray(inputs["W_hh"], np.float32)
    b = np.asarray(inputs["b_ih"], np.float32) + np.asarray(inputs["b_hh"], np.float32)
    W_fc = np.asarray(inputs["W_fc"], np.float32)
    b_fc = np.asarray(inputs["b_fc"], np.float32)

    # W_all: [64, 136] = [W_gat | W_gat@A_src | W_gat@A_dst]
    A_src = np.zeros((HEADS * C, HEADS), np.float32)
    A_dst = np.zeros((HEADS * C, HEADS), np.float32)
    for h in range(HEADS):
        A_src[h * C:(h + 1) * C, h] = att_src[h]
        A_dst[h * C:(h + 1) * C, h] = att_dst[h]
    W_all = np.concatenate([W_gat, W_gat @ A_src, W_gat @ A_dst], axis=1)
    kap = np.array([np.dot(W_edge[0, h * C:(h + 1) * C], att_edge[h])
                    for h in range(HEADS)], np.float32)
    kap_rep = np.broadcast_to(kap, (128, HEADS)).copy()
    gb_row = np.broadcast_to(gat_bias, (128, HEADS * C)).copy()
    iota128 = np.broadcast_to(np.arange(128, dtype=np.float32), (128, 128)).copy()
    # gate row order [f, i, o, g] (torch order is i,f,g,o)
    perm = np.concatenate([np.arange(32, 64), np.arange(0, 32),
                           np.arange(96, 128), np.arange(64, 96)])
    WihT = np.ascontiguousarray(W_ih[perm].T)
    WhhT = np.ascontiguousarray(W_hh[perm].T)
    br = np.ascontiguousarray(b[perm].reshape(128, 1))

    xbf = np.zeros((XPAD, F_IN), ml_dtypes.bfloat16)
    xbf[:TN] = x
    eaF = np.zeros((128, EAC), np.float32)
    j = np.arange(E)
    eaF[j % 128, j // 128] = ea

    src = ei[0].astype(np.int64)
    dst = ei[1].astype(np.int64)
    live = (dst >= D0) & (dst < D0 + DN)
    sl = src[live]
    dl = dst[live] - D0
    bkt = dl >> 7
    cnt = np.bincount(bkt, minlength=80)
    chb = int(max(1, -(-int(cnt.max()) // 128)))   # chunks per bucket
    nch = NB * chb

    by_bucket = {}
    order = np.argsort(bkt, kind="stable")
    pos = 0
    for gb in range(80):
        by_bucket[gb] = order[pos:pos + cnt[gb]]
        pos += cnt[gb]

    eal = ea[live]
    in_maps = []
    for k in range(NCORES):
        xI = np.full((128, nch), TN, np.int32)
        eaC = np.zeros((128, nch), np.float32)
        dstF = np.full((128, nch), -1.0, np.float32)
        ohT = np.zeros((128, nch * 128), ml_dtypes.bfloat16)
        for b_ in range(NB):
            gb = 10 * k - 1 + b_
            if not (0 <= gb < 80):
                continue
            sel = by_bucket[gb]
            ne = len(sel)
            assert ne <= chb * 128
            ii = np.arange(ne)
            cc = b_ * chb + ii // 128
            pp = ii % 128
            xI[pp, cc] = sl[sel].astype(np.int32)
            eaC[pp, cc] = eal[sel]
            dpos = (dl[sel] - 128 * gb).astype(np.int32)
            dstF[pp, cc] = dpos.astype(np.float32)
            ohT[dpos, cc * 128 + pp] = 1.0
        # own node features, transposed: bucket-local nodes [128*(10k-1), +1408)
        xTD = np.zeros((F_IN, NB * 128), ml_dtypes.bfloat16)
        lo = 128 * (10 * k - 1)
        for j2 in range(NB * 128):
            gn = lo + j2
            if 0 <= gn < DN:
                xTD[:, j2] = x[D0 + gn]
        Bwarm = np.zeros((128, NW), np.float32)
        if k == 0:
            Bwarm[32:96, :] = -30.0    # i and o gate rows of the junk warmup
        in_maps.append({
            "xnd": xbf, "eaF": eaF,
            "xI": xI, "ohT": ohT, "eaC": eaC, "dstF": dstF,
            "xTD": np.ascontiguousarray(xTD),
            "Bwarm": Bwarm.astype(ml_dtypes.bfloat16),
            "Wall": W_all.astype(ml_dtypes.bfloat16),
            "kap": kap_rep, "gbrow": gb_row, "iota": iota128,
            "Wih": WihT.astype(ml_dtypes.bfloat16),
            "Whh": WhhT.astype(ml_dtypes.bfloat16),
            "br": br,
            "Wfc": np.ascontiguousarray(W_fc.reshape(HID, 1)).astype(ml_dtypes.bfloat16),
            "bfc": np.ascontiguousarray(b_fc.reshape(1, 1)),
        })
    return in_maps, chb


def _build_nc(chb):
    nch = NB * chb
    nc = bacc.Bacc("TRN2", target_bir_lowering=False, debug=False,
                   num_devices=NCORES)
    g = lambda n, s, d=F32: nc.dram_tensor(n, s, d, kind="ExternalInput").ap()
    xnd = g("xnd", [XPAD, F_IN], BF16)
    eaF = g("eaF", [128, EAC])
    xI = g("xI", [128, nch], dt.int32)
    ohT = g("ohT", [128, nch * 128], BF16)
    eaC = g("eaC", [128, nch])
    dstF = g("dstF", [128, nch])
    xTD = g("xTD", [F_IN, NB * 128], BF16)
    Bwarm = g("Bwarm", [128, NW], BF16)
    Wall = g("Wall", [F_IN, 136], BF16)
    kap = g("kap", [128, HEADS])
    gbrow = g("gbrow", [128, 128])
    iota = g("iota", [128, 128])
    Wih = g("Wih", [128, 128], BF16)
    Whh = g("Whh", [HID, 128], BF16)
    br = g("br", [128, 1])
    Wfc = g("Wfc", [HID, 1], BF16)
    bfc = g("bfc", [1, 1])
    out = nc.dram_tensor("out", [1, SEQ], F32, kind="ExternalOutput").ap()

    with tile.TileContext(nc) as tc, ExitStack() as top:
        const = top.enter_context(tc.tile_pool(name="const", bufs=1))
        identB = const.tile([128, 128], BF16)
        make_identity(nc, identB[:])
        wall_t = const.tile([F_IN, 136], BF16); nc.sync.dma_start(wall_t[:], Wall[:])
        kap_t = const.tile([128, HEADS], F32); nc.sync.dma_start(kap_t[:], kap[:])
        gbr_t = const.tile([128, 128], F32); nc.sync.dma_start(gbr_t[:], gbrow[:])
        iota_t = const.tile([128, 128], F32); nc.sync.dma_start(iota_t[:], iota[:])
        wih_t = const.tile([128, 128], BF16); nc.sync.dma_start(wih_t[:], Wih[:])
        whh_t = const.tile([HID, 128], BF16); nc.sync.dma_start(whh_t[:], Whh[:])
        br_t = const.tile([128, 1], F32); nc.sync.dma_start(br_t[:], br[:])
        wfc_t = const.tile([HID, 1], BF16); nc.sync.dma_start(wfc_t[:], Wfc[:])
        bfc_t = const.tile([1, 1], F32); nc.sync.dma_start(bfc_t[:], bfc[:])
        bw_t = const.tile([128, NW], BF16); nc.sync.dma_start(bw_t[:], Bwarm[:])
        xi_t = const.tile([128, nch], dt.int32); nc.sync.dma_start(xi_t[:], xI[:])
        ohT_t = const.tile([128, nch * 128], BF16); nc.sync.dma_start(ohT_t[:], ohT[:])
        eac_t = const.tile([128, nch], F32); nc.sync.dma_start(eac_t[:], eaC[:])
        dsf_t = const.tile([128, nch], F32); nc.sync.dma_start(dsf_t[:], dstF[:])
        xtd_t = const.tile([F_IN, NB * 128], BF16); nc.sync.dma_start(xtd_t[:], xTD[:])
        meanr = const.tile([128, 1], F32)
        gatT = const.tile([128, NB * 128], BF16)     # [feat, bucket-local node]

        # ---------- Phase 0: mean(edge_attr), local full reduce ----------
        with ExitStack() as ph:
            sbm = ph.enter_context(tc.tile_pool(name="sbm", bufs=1))
            psm = ph.enter_context(tc.tile_pool(name="psm", bufs=1, space="PSUM"))
            eaf_t = sbm.tile([128, EAC], F32)
            nc.sync.dma_start(eaf_t[:], eaF[:])
            eap = sbm.tile([128, 1], F32)
            nc.vector.tensor_reduce(eap[:], eaf_t[:], mybir.AxisListType.X, ALU.add)
            onc = sbm.tile([128, 1], F32)
            nc.gpsimd.memset(onc[:], 1.0)
            ps1 = psm.tile([1, 1], F32, space="PSUM", tag="ps1")
            nc.tensor.matmul(ps1[:], lhsT=eap[:], rhs=onc[:], start=True, stop=True)
            eas = sbm.tile([1, 1], F32)
            nc.scalar.mul(eas[:], ps1[:], 1.0 / E)
            onr = sbm.tile([1, 128], F32)
            nc.gpsimd.memset(onr[:], 1.0)
            ps2 = psm.tile([128, 1], F32, space="PSUM", tag="ps2")
            nc.tensor.matmul(ps2[:], lhsT=onr[:], rhs=eas[:], start=True, stop=True)
            nc.vector.tensor_copy(meanr[:], ps2[:])

        # ---------- Phase 1: self tables (h|a_src|a_dst for own nodes) ----
        sfp = top.enter_context(tc.tile_pool(name="sfp", bufs=1))
        SF = sfp.tile([128, NB * 136], F32)
        SFv = SF[:].rearrange("p (j w) -> p j w", w=136)
        adB = sfp.tile([128, NB * 4], BF16)
        adBv = adB[:].rearrange("p (j w) -> p j w", w=4)
        selfSC = sfp.tile([128, NB * 132], BF16)
        sSCv = selfSC[:].rearrange("p (j w) -> p j w", w=132)
        with ExitStack() as ph:
            sbs = ph.enter_context(tc.tile_pool(name="sbs", bufs=1))
            pss = ph.enter_context(tc.tile_pool(name="pss", bufs=4, space="PSUM"))
            for b_ in range(NB):
                pf = pss.tile([128, 136], F32, space="PSUM", tag="pf")
                nc.tensor.matmul(pf[:], lhsT=xtd_t[:, b_ * 128:(b_ + 1) * 128],
                                 rhs=wall_t[:], start=True, stop=True)
                nc.vector.tensor_copy(SFv[:, b_, :], pf[:])
            nc.vector.tensor_copy(adBv, SFv[:, :, 132:136])
            QS = sbs.tile([128, NB * 4], F32)
            QSv = QS[:].rearrange("p (j w) -> p j w", w=4)
            nc.vector.tensor_tensor(out=QSv, in0=SFv[:, :, 128:132],
                                    in1=SFv[:, :, 132:136], op=ALU.add)
            kapb = kap_t[:].rearrange("p (o w) -> p o w", o=1) \
                .to_broadcast([128, NB, 4])
            nc.vector.scalar_tensor_tensor(out=QSv, in0=kapb, scalar=meanr[:],
                                           op0=ALU.mult, op1=ALU.add, in1=QSv)
            T2 = sbs.tile([128, NB * 4], F32)
            nc.vector.tensor_scalar_mul(T2[:], QS[:], LEAK)
            nc.vector.tensor_tensor(out=QS[:], in0=QS[:], in1=T2[:], op=ALU.max)
            SG1 = sbs.tile([128, NB * 4], F32)
            nc.scalar.activation(SG1[:], QS[:], AF.Sigmoid)
            nc.scalar.activation(T2[:], QS[:], AF.Sigmoid, scale=-1.0)
            nc.vector.reciprocal(T2[:], T2[:])
            SSf = sbs.tile([128, NB * 4], F32)
            nc.vector.tensor_tensor(out=SSf[:], in0=SG1[:], in1=T2[:], op=ALU.mult)
            ssf4 = SSf[:].rearrange("p (j h w) -> p j h w", h=HEADS, w=1) \
                .to_broadcast([128, NB, HEADS, C])
            sf4 = SFv[:, :, 0:128].rearrange("p j (h c) -> p j h c", h=HEADS)
            o4 = sSCv[:, :, 0:128].rearrange("p j (h c) -> p j h c", h=HEADS)
            nc.vector.tensor_tensor(out=o4, in0=sf4, in1=ssf4, op=ALU.mult)
            nc.vector.tensor_copy(sSCv[:, :, 128:132],
                                  SSf[:].rearrange("p (j w) -> p j w", w=4))

        # ---------- Phase 2: edge phase ----------
        NXT = (chb + 1) // 2          # transpose pair count per bucket
        with ExitStack() as ph:
            sbe = ph.enter_context(tc.tile_pool(name="sbe", bufs=2))
            sbq = ph.enter_context(tc.tile_pool(name="sbq", bufs=2))
            pse = ph.enter_context(tc.tile_pool(name="pse", bufs=2, space="PSUM"))
            psh = ph.enter_context(tc.tile_pool(name="psh", bufs=2, space="PSUM"))
            psa = ph.enter_context(tc.tile_pool(name="psa", bufs=2, space="PSUM"))
            psk = ph.enter_context(tc.tile_pool(name="psk", bufs=2, space="PSUM"))
            for b_ in range(NB):
                c0 = b_ * chb
                XGBt = sbe.tile([128, chb * 64], BF16, tag="XGB")
                XGBv = XGBt[:].rearrange("p (e w) -> p e w", w=64)
                for cc in range(chb):
                    nc.gpsimd.indirect_dma_start(
                        out=XGBv[:, cc, :], out_offset=None, in_=xnd[:],
                        in_offset=bass.IndirectOffsetOnAxis(
                            ap=xi_t[:, c0 + cc:c0 + cc + 1], axis=0))
                XGB = XGBt[:]
                XT = sbe.tile([F_IN, chb * 128], BF16, tag="XT")
                for pr in range(NXT):
                    w = min(128, chb * 64 - pr * 128)
                    pxT = psk.tile([128, 128], BF16, space="PSUM", tag="ptr")
                    nc.tensor.transpose(out=pxT[0:w, :],
                                        in_=XGB[:, pr * 128:pr * 128 + w],
                                        identity=identB[:])
                    nc.vector.tensor_copy(XT[:, (2 * pr) * 128:(2 * pr + 1) * 128],
                                          pxT[0:64, :])
                    if w > 64:
                        nc.vector.tensor_copy(
                            XT[:, (2 * pr + 1) * 128:(2 * pr + 2) * 128],
                            pxT[64:128, :])
                # h|a_src per edge; batches of 3 chunks share one PSUM tile
                hEs = sbe.tile([128, chb * 132], BF16, tag="hEs")
                hEv = hEs[:].rearrange("p (e w) -> p e w", w=132)
                Qb = sbq.tile([128, chb * 4], F32, tag="Qb")
                Qv = Qb[:].rearrange("p (e w) -> p e w", w=4)
                nb3 = (chb + 2) // 3
                for b3 in range(nb3):
                    n3 = min(3, chb - b3 * 3)
                    phE = psh.tile([128, 3 * 132], F32, space="PSUM", tag="phE")
                    for j3 in range(n3):
                        cc = b3 * 3 + j3
                        lhs = XT[:, cc * 128:(cc + 1) * 128]
                        nc.tensor.matmul(phE[:, j3 * 132:(j3 + 1) * 132],
                                         lhsT=lhs, rhs=wall_t[:, 0:132],
                                         start=True, stop=True)
                    phEv = phE[:].rearrange("p (e w) -> p e w", w=132)
                    nc.vector.tensor_copy(hEv[:, b3 * 3:b3 * 3 + n3, :],
                                          phEv[:, 0:n3, :])
                    nc.vector.tensor_copy(Qv[:, b3 * 3:b3 * 3 + n3, :],
                                          phEv[:, 0:n3, 128:132])
                # one-hots for the whole bucket in one op
                ohs = sbe.tile([128, chb * 128], BF16, tag="ohs")
                ohv = ohs[:].rearrange("p (e w) -> p e w", w=128)
                dfb = dsf_t[:, c0:c0 + chb].rearrange("p (e w) -> p e w", w=1) \
                    .to_broadcast([128, chb, 128])
                iob = iota_t[:].rearrange("p (o w) -> p o w", o=1) \
                    .to_broadcast([128, chb, 128])
                nc.vector.tensor_tensor(out=ohv, in0=dfb, in1=iob, op=ALU.is_equal)
                # a_dst per edge: one-hot-transpose (host-shipped) matmul
                padc = psa.tile([128, chb * 4], F32, space="PSUM", tag="padc")
                for cc in range(chb):
                    nc.tensor.matmul(
                        padc[:, cc * 4:(cc + 1) * 4],
                        lhsT=ohT_t[:, (c0 + cc) * 128:(c0 + cc + 1) * 128],
                        rhs=adBv[:, b_, :], start=True, stop=True)
                # q = a_src + a_dst + ea*kap ; s = exp(leaky_relu(q))
                kmb = sbq.tile([128, chb * 4], F32, tag="kmb")
                kmv = kmb[:].rearrange("p (e w) -> p e w", w=4)
                eab = eac_t[:, c0:c0 + chb].rearrange("p (e w) -> p e w", w=1) \
                    .to_broadcast([128, chb, 4])
                kab = kap_t[:].rearrange("p (o w) -> p o w", o=1) \
                    .to_broadcast([128, chb, 4])
                nc.vector.tensor_tensor(out=kmv, in0=eab, in1=kab, op=ALU.mult)
                nc.vector.tensor_tensor(out=Qb[:], in0=Qb[:], in1=kmb[:], op=ALU.add)
                nc.vector.tensor_tensor(out=Qb[:], in0=Qb[:], in1=padc[:],
                                        op=ALU.add)
                nc.vector.tensor_scalar_mul(kmb[:], Qb[:], LEAK)
                nc.vector.tensor_tensor(out=Qb[:], in0=Qb[:], in1=kmb[:], op=ALU.max)
                SG = sbq.tile([128, chb * 4], F32, tag="SG")
                nc.scalar.activation(SG[:], Qb[:], AF.Sigmoid)
                nc.scalar.activation(kmb[:], Qb[:], AF.Sigmoid, scale=-1.0)
                nc.vector.reciprocal(kmb[:], kmb[:])
                Sbf = sbq.tile([128, chb * 4], BF16, tag="Sbf")
                nc.vector.tensor_tensor(out=Sbf[:], in0=SG[:], in1=kmb[:], op=ALU.mult)
                Sbv = Sbf[:].rearrange("p (e w) -> p e w", w=4)
                # messages and scatter
                SCb = sbe.tile([128, chb * 132], BF16, tag="SCb")
                SCv = SCb[:].rearrange("p (e w) -> p e w", w=132)
                sb4 = Sbf[:].rearrange("p (e h w) -> p e h w", h=HEADS, w=1) \
                    .to_broadcast([128, chb, HEADS, C])
                he4 = hEv[:, :, 0:128].rearrange("p e (h c) -> p e h c", h=HEADS)
                sc4 = SCv[:, :, 0:128].rearrange("p e (h c) -> p e h c", h=HEADS)
                nc.vector.tensor_tensor(out=sc4, in0=he4, in1=sb4, op=ALU.mult)
                nc.vector.tensor_copy(SCv[:, :, 128:132], Sbv)
                pacc = pse.tile([128, 132], F32, space="PSUM", tag="pacc")
                for cc in range(chb):
                    nc.tensor.matmul(pacc[:], lhsT=ohv[:, cc, :], rhs=SCv[:, cc, :],
                                     start=(cc == 0), stop=(cc == chb - 1))
                # add self loops, normalize, bias, relu, transpose
                nc.vector.tensor_tensor(out=pacc[:], in0=pacc[:],
                                        in1=sSCv[:, b_, :], op=ALU.add)
                dn = sbq.tile([128, 4], F32, tag="dn")
                nc.vector.tensor_scalar_add(dn[:], pacc[:, 128:132], 1e-16)
                nc.vector.reciprocal(dn[:], dn[:])
                gn = sbq.tile([128, 128], F32, tag="gn")
                g4 = gn[:].rearrange("p (h c) -> p h c", h=HEADS)
                p4 = pacc[:, 0:128].rearrange("p (h c) -> p h c", h=HEADS)
                d4 = dn[:].rearrange("p (h w) -> p h w", w=1) \
                    .to_broadcast([128, HEADS, C])
                nc.vector.tensor_tensor(out=g4, in0=p4, in1=d4, op=ALU.mult)
                nc.vector.tensor_tensor(out=gn[:], in0=gn[:], in1=gbr_t[:], op=ALU.add)
                gnb = sbq.tile([128, 128], BF16, tag="gnb")
                nc.vector.tensor_scalar_max(gnb[:], gn[:], 0.0)
                pgT = psk.tile([128, 128], BF16, space="PSUM", tag="ptr")
                nc.tensor.transpose(out=pgT[:], in_=gnb[:], identity=identB[:])
                nc.vector.tensor_copy(gatT[:, b_ * 128:(b_ + 1) * 128], pgT[:])

        # ---------- Phase 3: gx = Wih.T @ gatT (+ warmup mask), bf16 ----
        persist = top.enter_context(tc.tile_pool(name="persist", bufs=1))
        Gxb = persist.tile([128, NTL], BF16)
        H = persist.tile([HID, NTL], BF16)
        nc.gpsimd.memset(H[:], 0.0)
        GSL = [(0, 512), (512, 1024), (1024, NTL)]
        with ExitStack() as ph:
            psg = ph.enter_context(tc.tile_pool(name="psg", bufs=3, space="PSUM"))
            for lo, hi in GSL:
                pgx = psg.tile([128, 512], F32, space="PSUM", tag="pgx")
                nc.tensor.matmul(pgx[:, 0:hi - lo], lhsT=wih_t[:],
                                 rhs=gatT[:, 32 + lo:32 + hi],
                                 start=True, stop=True)
                nc.vector.tensor_copy(Gxb[:, lo:hi], pgx[:, 0:hi - lo])
            nc.vector.tensor_tensor(out=Gxb[:, 0:NW], in0=Gxb[:, 0:NW],
                                    in1=bw_t[:], op=ALU.add)

        # ---------- Phase 4: LSTM fixed point ----------
        # One chained [32, NTL] sequence; warmup absorbs the halo boundary.
        # gx lands in PSUM via a PE identity matmul so the Whh matmul can
        # accumulate onto it within a normal PE accumulation group.
        YS = persist.tile([HID, SEQ], BF16)
        with ExitStack() as ph:
            sbl = ph.enter_context(tc.tile_pool(name="sbl", bufs=2))
            psl = ph.enter_context(tc.tile_pool(name="psl", bufs=2, space="PSUM"))
            for it in range(ITERS):
                pG = psl.tile([128, 2048], F32, space="PSUM", tag="pG")
                for lo, hi in GSL:
                    nc.tensor.matmul(pG[:, lo:hi], lhsT=identB[:],
                                     rhs=Gxb[:, lo:hi], start=True,
                                     stop=(it == 0))
                if it > 0:
                    for lo, hi in GSL:
                        nc.tensor.matmul(pG[:, lo:hi], lhsT=whh_t[:],
                                         rhs=H[:, lo:hi], start=False, stop=True)
                S_ = sbl.tile([96, NTL], BF16, tag="S")
                nc.scalar.activation(S_[:], pG[0:96, 0:NTL], AF.Sigmoid,
                                     bias=br_t[0:96, :])
                Tg = sbl.tile([64, NTL], BF16, tag="Tg")
                nc.scalar.activation(Tg[32:64, :], pG[96:128, 0:NTL], AF.Tanh,
                                     bias=br_t[96:128, :])
                Zt = sbl.tile([HID, NTL], BF16, tag="Zt")
                nc.vector.tensor_tensor(out=Zt[:], in0=S_[32:64, :],
                                        in1=Tg[32:64, :], op=ALU.mult)
                Ct = sbl.tile([HID, NTL], BF16, tag="Ct")
                nc.vector.tensor_tensor_scan(out=Ct[:], data0=S_[0:32, :],
                                             data1=Zt[:], initial=0.0,
                                             op0=ALU.mult, op1=ALU.add)
                TCu = sbl.tile([96, NTL], BF16, tag="TCu")
                nc.scalar.activation(TCu[64:96, :], Ct[:], AF.Tanh)
                if it < ITERS - 1:
                    nc.vector.tensor_tensor(
                        out=H[0:32, 1:NTL],
                        in0=S_[64:96, 0:NTL - 1],
                        in1=TCu[64:96, 0:NTL - 1], op=ALU.mult)
                else:
                    nc.vector.tensor_tensor(
                        out=YS[:], in0=S_[64:96, NW:NTL],
                        in1=TCu[64:96, NW:NTL], op=ALU.mult)

        # ---------- Phase 5: FC ----------
        with ExitStack() as ph:
            sbf = ph.enter_context(tc.tile_pool(name="sbf", bufs=1))
            psf = ph.enter_context(tc.tile_pool(name="psf", bufs=4, space="PSUM"))
            OF = sbf.tile([1, SEQ], F32)
            for q in range(4):
                pf = psf.tile([1, 320], F32, space="PSUM", tag="pfc")
                nc.tensor.matmul(pf[:], lhsT=wfc_t[:],
                                 rhs=YS[:, q * 320:(q + 1) * 320],
                                 start=True, stop=True)
                nc.vector.tensor_scalar_add(OF[:, q * 320:(q + 1) * 320],
                                            pf[:], bfc_t[:])
            nc.sync.dma_start(out[:], OF[:])

    nc.compile()
    return nc


def run(inputs, trace=False):
    in_maps, chb = _prep_host(inputs)
    if chb not in _CACHE:
        _CACHE[chb] = _build_nc(chb)
    nc = _CACHE[chb]
    res = run_bass_kernel_spmd(nc, in_maps, list(range(NCORES)), trace=trace)
    return res


def kernel(**inputs) -> np.ndarray:
    res = run(inputs)
    full = np.concatenate([np.asarray(res.results[k]["out"][0], np.float32)
                           for k in range(NCORES)])
    return np.ascontiguousarray(full[:N].reshape(N, 1))
